# revision 5
# baseline (speedup 1.0000x reference)
"""ConvCapsuleLayer TRN2 kernel v9.  HW: ~771 us (baseline v2: 1797 us).

Sharding: 8 cores = B(2) x D-quarters(4); zero cross-core communication.
All PE operands fp16 (full-rate 1 cyc/col streaming - fp32r measured at
4 cyc/col on HW); PSUM accumulation fp32. Conv kd-MIXED packing: all 27
(kd,kh,kw) taps baked into the contraction dim as 3x K=128 + 1x K=48
matmuls per 384-col chunk (4 MMs vs v2's 6); shifts/plane-selection done
host-side into [DPC,IC,3,128,2500]+[DPC,IC,48,2500] fp16 DRAM tiles.
Routing (per half-plane, 1152 cols):
 - reduce+broadcast fused into single 0/1-mask MMs (ena8bc/esbc) so squash
   and softmax stats come out pre-replicated across partitions,
 - logits live in a PSUM bank per chunk; iter2 delta MMs accumulate on top
   of iter1's (start=False) - no logit copies/adds,
 - exp operands fp32 (iter-3 logits reach ~21; fp16 exp overflows at 11.1),
 - elementwise fp16 half-plane-wide (DVE 2x mode); squares + ic2/3 route
   muls on DVE, route-bcast for ic0/1 consumed straight from PSUM,
 - vote-sum + per-partition bias fused into one scalar_tensor_tensor,
 - no PE transposes: output stays [OC, pos] fp16; host transposes/casts.
Tried and reverted (all regressed): gpsimd offload (TT 2437ns vs DVE
800ns), SBUF-resident logits (+copy/add ops), interleaved conv/routing
emission order (ACT/PSUM contention).
"""
import sys
import numpy as np

sys.path.insert(0, "/opt/trn_rl_repo")

import concourse.bass as bass
import concourse.mybir as mybir
from concourse import bacc, tile
from contextlib import ExitStack

F32 = mybir.dt.float32
F16 = mybir.dt.float16
AF = mybir.ActivationFunctionType
ALU = mybir.AluOpType

B, D, H, W, IC, A = 2, 24, 48, 48, 4, 16
NC, NA = 8, 16
OC = 128
DPC = 6
DSLAB = DPC + 2
HP = 50
PLANE_POS = H * W      # 2304
CN = 384               # chunk cols (8 h-rows)
NCH = PLANE_POS // CN  # 6 chunks per plane
CROWS = CN // W        # 8
HPL = PLANE_POS // 2   # 1152, half-plane
NCH2 = HPL // CN       # 3 chunks per half-plane


def build_program(use_gpsimd=False):
    nc = bacc.Bacc("TRN2", target_bir_lowering=False, debug=False, num_devices=8)
    xg_e = nc.dram_tensor("xg", [DPC, IC, 3, OC, HP * HP], F16, kind="ExternalInput").ap()
    xgc_e = nc.dram_tensor("xgc", [DPC, IC, 48, HP * HP], F16, kind="ExternalInput").ap()
    wg_e = nc.dram_tensor("wg", [3, OC, OC], F16, kind="ExternalInput").ap()
    wgc_e = nc.dram_tensor("wgc", [OC, OC], F16, kind="ExternalInput").ap()
    bias_e = nc.dram_tensor("bias", [OC, 1], F32, kind="ExternalInput").ap()
    bias8_e = nc.dram_tensor("bias8", [OC, 1], F32, kind="ExternalInput").ap()
    masks_e = nc.dram_tensor("masks", [10, OC, OC], F16, kind="ExternalInput").ap()
    esbc32_e = nc.dram_tensor("esbc32", [OC, OC], F32, kind="ExternalInput").ap()
    out_e = nc.dram_tensor("out", [DPC, OC, PLANE_POS], F16, kind="ExternalOutput").ap()

    with ExitStack() as ctx:
        tc = ctx.enter_context(tile.TileContext(nc))
        cpool = ctx.enter_context(tc.tile_pool(name="const", bufs=1))
        planep = ctx.enter_context(tc.tile_pool(name="planes", bufs=2))
        votesp = ctx.enter_context(tc.tile_pool(name="votes", bufs=2))
        s16 = ctx.enter_context(tc.tile_pool(name="s16", bufs=2))
        s32 = ctx.enter_context(tc.tile_pool(name="s32", bufs=2))
        ps_conv = ctx.enter_context(tc.tile_pool(name="ps_conv", bufs=2, space="PSUM"))
        ps_L = ctx.enter_context(tc.tile_pool(name="ps_L", bufs=3, space="PSUM"))
        ps_bc = ctx.enter_context(tc.tile_pool(name="ps_bc", bufs=3, space="PSUM"))

        # --- resident constants ---
        wg_s = cpool.tile([OC, 3, OC], F16, tag="wg")
        nc.sync.dma_start(out=wg_s[:], in_=wg_e.rearrange("k p m -> p k m"))
        wgc_s = cpool.tile([OC, OC], F16, tag="wgc")
        nc.sync.dma_start(out=wgc_s[:], in_=wgc_e[:])
        bias_s = cpool.tile([OC, 1], F32, tag="bias")
        nc.sync.dma_start(out=bias_s[:], in_=bias_e[:])
        bias8_s = cpool.tile([OC, 1], F32, tag="bias8")
        nc.sync.dma_start(out=bias8_s[:], in_=bias8_e[:])
        mk_s = cpool.tile([OC, 10, OC], F16, tag="masks")
        nc.sync.dma_start(out=mk_s[:], in_=masks_e.rearrange("k p m -> p k m"))
        esbc32_s = cpool.tile([OC, OC], F32, tag="esbc32")
        nc.sync.dma_start(out=esbc32_s[:], in_=esbc32_e[:])
        esbc = mk_s[:, 0, :]
        ena8bc = mk_s[:, 1, :]
        erbc = [mk_s[:, 2 + i, :] for i in range(IC)]
        edl = [mk_s[:, 6 + i, :] for i in range(IC)]

        eng2 = nc.gpsimd if use_gpsimd else nc.vector

        def squash_fac(pre, nrm_scale, t1_scale, sq_bias=0.0):
            """pre [OC,HPL] f16 -> fac [OC,HPL] f16 (replicated), using
            nb = sum_na (pre + sq_bias)^2 per capsule via ena8bc MM.
            fac = sqrt(nrm_scale*nb) / (1 + t1_scale*nb)."""
            sq = s16.tile([OC, HPL], F16, tag="sq", bufs=3)
            nc.scalar.activation(out=sq[:], in_=pre, func=AF.Square, bias=sq_bias)
            nbs = []
            for c in range(NCH2):
                nb = ps_bc.tile([OC, CN], F32, tag="bc", name=f"nb{c}")
                nc.tensor.matmul(out=nb[:], lhsT=ena8bc, rhs=sq[:, c * CN:(c + 1) * CN],
                                 start=True, stop=True)
                nbs.append(nb)
            nrm = s16.tile([OC, HPL], F16, tag="nrm")
            for c in range(NCH2):
                nc.scalar.activation(out=nrm[:, c * CN:(c + 1) * CN], in_=nbs[c][:],
                                     func=AF.Sqrt, scale=nrm_scale)
            t1 = s32.tile([OC, HPL], F32, tag="t1")
            for c in range(NCH2):
                nc.scalar.activation(out=t1[:, c * CN:(c + 1) * CN], in_=nbs[c][:],
                                     func=AF.Identity, bias=1.0, scale=t1_scale)
            nc.vector.reciprocal_approx_fast(out=t1[:], in_=t1[:])
            fac = s16.tile([OC, HPL], F16, tag="fac")
            nc.vector.tensor_mul(out=fac[:], in0=nrm[:], in1=t1[:])
            return fac

        def softmax_r(Ls):
            """Ls: list of 3 PSUM logit banks [OC,CN] (L32 layout) ->
            r [OC,HPL] f16 (normalized route, L32 layout)."""
            e = s32.tile([OC, HPL], F32, tag="e")
            for c in range(NCH2):
                nc.scalar.activation(out=e[:, c * CN:(c + 1) * CN], in_=Ls[c][:],
                                     func=AF.Exp)
            rs = s32.tile([OC, HPL], F32, tag="rs")
            for c in range(NCH2):
                sb = ps_bc.tile([OC, CN], F32, tag="bc")
                nc.tensor.matmul(out=sb[:], lhsT=esbc32_s[:], rhs=e[:, c * CN:(c + 1) * CN],
                                 start=True, stop=True)
                nc.vector.reciprocal_approx_fast(
                    out=rs[:, c * CN:(c + 1) * CN], in_=sb[:])
            r = s16.tile([OC, HPL], F16, tag="r")
            nc.vector.tensor_mul(out=r[:], in0=e[:], in1=rs[:])
            return r

        def weighted_pre(vch, r):
            """pre = sum_ic bcast_ic(r)*votes_ic + bias, [OC,HPL] f16."""
            ps = []
            for i in range(IC):
                p = s16.tile([OC, HPL], F16, tag=f"p{i}", name=f"p{i}")
                if i < 2:
                    for c in range(NCH2):
                        rb = ps_bc.tile([OC, CN], F32, tag="bc", name=f"rb{i}{c}")
                        nc.tensor.matmul(out=rb[:], lhsT=erbc[i],
                                         rhs=r[:, c * CN:(c + 1) * CN],
                                         start=True, stop=True)
                        nc.vector.tensor_mul(out=p[:, c * CN:(c + 1) * CN],
                                             in0=vch[i][:, c * CN:(c + 1) * CN],
                                             in1=rb[:])
                else:
                    rbs = s16.tile([OC, HPL], F16, tag=f"rbs{i}", name=f"rbs{i}")
                    for c in range(NCH2):
                        rb = ps_bc.tile([OC, CN], F32, tag="bc", name=f"rb{i}{c}")
                        nc.tensor.matmul(out=rb[:], lhsT=erbc[i],
                                         rhs=r[:, c * CN:(c + 1) * CN],
                                         start=True, stop=True)
                        nc.scalar.copy(out=rbs[:, c * CN:(c + 1) * CN], in_=rb[:])
                    eng2.tensor_mul(out=p[:], in0=vch[i], in1=rbs[:])
                ps.append(p)
            a01 = s16.tile([OC, HPL], F16, tag="a01")
            eng2.tensor_add(out=a01[:], in0=ps[0][:], in1=ps[1][:])
            a23 = s16.tile([OC, HPL], F16, tag="a23")
            nc.vector.tensor_add(out=a23[:], in0=ps[2][:], in1=ps[3][:])
            pre = s16.tile([OC, HPL], F16, tag="pre")
            nc.vector.scalar_tensor_tensor(out=pre[:], in0=a01[:],
                                           scalar=bias_s[:], in1=a23[:],
                                           op0=ALU.add, op1=ALU.add)
            return pre

        def delta_accum(vch, act, Ls, first):
            """L_c += per-ic sum_na votes*act (edl MMs, L32 layout)."""
            ds = []
            for i in range(IC):
                d = s16.tile([OC, HPL], F16, tag=f"p{i}", name=f"d{i}")
                eng = nc.vector if i < 2 else eng2
                eng.tensor_mul(out=d[:], in0=vch[i], in1=act[:])
                ds.append(d)
            for c in range(NCH2):
                for i in range(IC):
                    nc.tensor.matmul(out=Ls[c][:], lhsT=edl[i],
                                     rhs=ds[i][:, c * CN:(c + 1) * CN],
                                     start=(first and i == 0), stop=(i == IC - 1))

        # ===================== main loop =====================
        for dp in range(DPC):
            vts = []
            for n in range(IC):
                xt = planep.tile([OC, 3, HP * HP], F16, tag="xt")
                nc.sync.dma_start(out=xt[:], in_=xg_e[dp, n].rearrange("g p m -> p g m"))
                xc = planep.tile([48, HP * HP], F16, tag="xc")
                nc.sync.dma_start(out=xc[:], in_=xgc_e[dp, n])
                v = votesp.tile([OC, PLANE_POS], F16, tag=f"v{n}")
                vts.append(v)
                for c in range(NCH):
                    h0 = c * CROWS
                    pc = ps_conv.tile([OC, CN], F32, tag="conv")
                    off = h0 * HP + 1
                    tA = xt[:]
                    for g in range(3):
                        rhsA = bass.AP(tA.tensor, tA.offset + g * HP * HP + off,
                                       [list(tA.ap[0]), [HP, CROWS], [1, 48]])
                        nc.tensor.matmul(out=pc[:], lhsT=wg_s[:, g, :],
                                         rhs=rhsA, start=(g == 0), stop=False)
                    tC = xc[0:48, :]
                    rhsC = bass.AP(tC.tensor, tC.offset + off,
                                   [list(tC.ap[0]), [HP, CROWS], [1, 48]])
                    nc.tensor.matmul(out=pc[:], lhsT=wgc_s[0:48, :],
                                     rhs=rhsC, start=False, stop=True)
                    nc.scalar.copy(out=v[:, c * CN:(c + 1) * CN], in_=pc[:])

            for h in range(2):
                hs = h * HPL
                vch = [vts[n][:, hs:hs + HPL] for n in range(IC)]
                # ---- iter 1: uniform route; P = sum votes + 8*bias ----
                # P = sum votes (unbiased); the +8*bias rides inside the ACT
                # Square (squash stats) and the act1 STT.
                t01 = s16.tile([OC, HPL], F16, tag="t01", bufs=3)
                nc.gpsimd.tensor_add(out=t01[:], in0=vch[0], in1=vch[1])
                t23 = s16.tile([OC, HPL], F16, tag="t23", bufs=3)
                nc.gpsimd.tensor_add(out=t23[:], in0=vch[2], in1=vch[3])
                P = s16.tile([OC, HPL], F16, tag="P")
                nc.gpsimd.tensor_add(out=P[:], in0=t01[:], in1=t23[:])
                # pre1_true = (P+8b)/8: fold 1/64 into t1-scale, (1/8)*(1/8)
                # into nrm-scale (sqrt(nb/4096) = sqrt(n2)/8)
                fac1 = squash_fac(P[:], 1.0 / 4096, 1.0 / 64, sq_bias=bias8_s[:])
                act1 = s16.tile([OC, HPL], F16, tag="act")
                nc.vector.scalar_tensor_tensor(out=act1[:], in0=P[:],
                                               scalar=bias8_s[:], in1=fac1[:],
                                               op0=ALU.add, op1=ALU.mult)
                Ls = [ps_L.tile([OC, CN], F32, tag="L", name=f"L{c}")
                      for c in range(NCH2)]
                delta_accum(vch, act1[:], Ls, first=True)
                # ---- iter 2 ----
                r2 = softmax_r(Ls)
                pre2 = weighted_pre(vch, r2)
                fac2 = squash_fac(pre2[:], 1.0, 1.0)
                act2 = s16.tile([OC, HPL], F16, tag="act")
                nc.vector.tensor_mul(out=act2[:], in0=pre2[:], in1=fac2[:])
                delta_accum(vch, act2[:], Ls, first=False)
                # ---- iter 3 ----
                r3 = softmax_r(Ls)
                pre3 = weighted_pre(vch, r3)
                fac3 = squash_fac(pre3[:], 1.0, 1.0)
                o = s16.tile([OC, HPL], F16, tag="o")
                nc.vector.tensor_mul(out=o[:], in0=pre3[:], in1=fac3[:])
                nc.sync.dma_start(out=out_e[dp][:, hs:hs + HPL], in_=o[:])

    nc.compile()
    return nc


# ===================== host side =====================

def prep_inputs(x, conv_w, b):
    x = np.asarray(x, np.float32)
    conv_w = np.asarray(conv_w, np.float32)
    b = np.asarray(b, np.float32)

    wg = np.zeros((3, OC, OC), np.float32)
    wgc = np.zeros((OC, OC), np.float32)
    for t in range(27):
        kd, kh, kw = t // 9, (t % 9) // 3, t % 3
        blk = conv_w[:, :, kd, kh, kw].T  # [16(a), OC]
        if t < 24:
            wg[t // 8, 16 * (t % 8):16 * (t % 8) + 16] = blk
        else:
            wgc[16 * (t - 24):16 * (t - 24) + 16] = blk
    wg = wg.astype(np.float16)
    wgc = wgc.astype(np.float16)

    bias = b[0, 0, 0].reshape(OC, 1).astype(np.float32)
    bias8 = (8.0 * bias).astype(np.float32)

    # masks: [esbc, ena8bc, erbc0-3, edl0-3], each [OC(part) x OC(out)]
    masks = np.zeros((10, OC, OC), np.float32)
    for i in range(IC):
        for n in range(NC):
            for k in range(32):
                masks[0, 32 * i + n, 32 * i + k] = 1.0      # esbc
    for ncp in range(NC):
        for na in range(NA):
            for na2 in range(NA):
                masks[1, 16 * ncp + na, 16 * ncp + na2] = 1.0   # ena8bc
    for i in range(IC):
        for n in range(NC):
            for na in range(NA):
                masks[2 + i, 32 * i + n, 16 * n + na] = 1.0     # erbc_i
    for i in range(IC):
        for ncp in range(NC):
            for na in range(NA):
                for j in range(4):
                    masks[6 + i, 16 * ncp + na, 32 * i + 8 * j + ncp] = 1.0  # edl_i
    masks = masks.astype(np.float16)

    xt = np.transpose(x, (0, 4, 5, 1, 2, 3))  # [B, ICg, A, D, H, W]

    from numpy.lib.stride_tricks import sliding_window_view

    in_maps = []
    for core in range(8):
        bc, dq = core // 4, core % 4
        d0 = dq * DPC - 1
        xg = np.zeros((DPC, IC, 3, OC, HP * HP), np.float16)
        xgc = np.zeros((DPC, IC, 48, HP * HP), np.float16)
        for ic in range(IC):
            n_g = 4 * bc + ic
            bp, icp = n_g % 2, n_g // 2
            xpad = np.zeros((A, DSLAB, 52, 52), np.float32)
            lo, hi = max(0, d0), min(D, d0 + DSLAB)
            xpad[:, lo - d0:hi - d0, 1:49, 2:50] = xt[bp, icp, :, lo:hi]
            # win[a, s, kh, kw] = xpad[a, s, kh:kh+50, kw:kw+50]
            win = sliding_window_view(xpad, (HP, HP), axis=(2, 3))
            for t in range(27):
                kd, kh, kw = t // 9, (t % 9) // 3, t % 3
                # [A, DPC, 50, 50] -> [DPC, A, 2500]
                blk = win[:, kd:kd + DPC, kh, kw].transpose(1, 0, 2, 3).reshape(
                    DPC, A, HP * HP).astype(np.float16)
                if t < 24:
                    g, j = t // 8, t % 8
                    xg[:, ic, g, 16 * j:16 * j + 16] = blk
                else:
                    xgc[:, ic, 16 * (t - 24):16 * (t - 24) + 16] = blk
        in_maps.append(dict(xg=xg, xgc=xgc, wg=wg, wgc=wgc, bias=bias,
                            bias8=bias8, masks=masks,
                            esbc32=masks[0].astype(np.float32)))
    return in_maps


def assemble_output(results):
    out = np.zeros((B, D, H, W, NC, NA), np.float32)
    for core in range(8):
        bc, dq = core // 4, core % 4
        r = results[core]["out"].astype(np.float32)  # [DPC, OC, 2304]
        r = r.reshape(DPC, NC, NA, H, W).transpose(0, 3, 4, 1, 2)
        out[bc, dq * DPC:(dq + 1) * DPC] = r
    return out


_NC_PROG = None


def _get_prog():
    global _NC_PROG
    if _NC_PROG is None:
        _NC_PROG = build_program()
    return _NC_PROG


def kernel(x, conv_w, b):
    """Full (unsharded) inputs -> full output [2, 24, 48, 48, 8, 16] fp32."""
    from concourse.bass_utils import run_bass_kernel_spmd
    nc = _get_prog()
    in_maps = prep_inputs(x, conv_w, b)
    res = run_bass_kernel_spmd(nc, in_maps, list(range(8)))
    return assemble_output(res.results).astype(np.float32)


def run_traced(x, conv_w, b):
    """Like kernel() but with NTFF tracing; returns (output, BassKernelResults)."""
    try:
        import antenv.axon_hooks as ah
        from trn_agent_boot.trn_boot import _ntff_profile_via_ctypes
        if ah.get_axon_ntff_profile_hook() is None:
            ah.set_axon_ntff_profile_hook(
                _ntff_profile_via_ctypes("/opt/axon/libaxon_pjrt.so"))
    except Exception:
        pass
    from concourse.bass_utils import run_bass_kernel_spmd
    nc = _get_prog()
    in_maps = prep_inputs(x, conv_w, b)
    res = run_bass_kernel_spmd(nc, in_maps, list(range(8)), trace=True)
    return assemble_output(res.results).astype(np.float32), res

